# revision 1
# baseline (speedup 1.0000x reference)
"""Trainium2 Bass kernel: multi-head self-attention block (dense transformer).

Reference computation (fp32):
    qkv = x @ w_qkv + b_qkv                  # x [b, n, dim], w_qkv [dim, 3*dim]
    q, k, v = split(qkv); heads = 16, dh = 64
    dots = (q @ k^T) * dim**-0.5  (per head)
    attn = softmax(dots, axis=-1)
    out  = (attn @ v) @ w_out + b_out        # [b, n, dim]

Sharding (8 cores): data-parallel over batch (b=2) x tensor-parallel over
head-groups (4 groups of 4 heads).  core c -> batch c//4, head-group c%4.
Each core computes q/k/v for its 4 heads only, runs attention, and multiplies
by its 256-row slice of w_out, producing a partial [n, dim] output.  The host
sums the 4 partials per batch (the "all-reduce") and adds b_out.

Device layout choices (per core):
  - host supplies x_b^T (feature-major) so no on-device transpose is needed;
    an extra ones-row k-tile folds b_qkv into the projection matmul.
  - qT, kT are computed feature-major [256, n]; v token-major [n, 256].
  - scores are computed TRANSPOSED (S^T [j, i]) so that attn@v needs no
    transpose; the two heads of a pair run concurrently on the PE via
    row-tiling (K=64 each).
  - softmax: exp on the scalar engine (PSUM -> SBUF).  attn@v uses the
    stationary operand [v_h | ones] (M=128): one matmul yields both the
    unnormalized attention output (rows 0-63) and the denominator Z
    replicated across rows 64-127, so normalization is a plain elementwise
    multiply on DVE (no partition broadcast needed).
  - ALL matmuls use moving-dim N=256: measured fp32r throughput on TRN2 is
    ~126 ns/mm at N=256 vs ~330 ns at N=512 (2.6x per column).
  - attn@v runs one j-block behind scores/exp (software pipeline) so the PE
    always has independent work while the scalar engine computes exp.
All matmul operands live in float32r tensors (single-pass PE mode;
~1.5e-4 relative rounding vs fp32).
"""

import numpy as np

import concourse.bacc as bacc
import concourse.mybir as mybir
import concourse.tile as tile
from concourse.bass_utils import run_bass_kernel_spmd

P = 128
DIM = 1024
HEADS = 16
B = 2
N = 2048
NCORES = 8
HGROUPS = 4                     # head-groups (tensor parallel)
H_LOC = HEADS // HGROUPS        # 4 heads per core
DH = DIM // HEADS               # 64
F_LOC = H_LOC * DH              # 256 features per core (per q/k/v)
SCALE = DIM ** -0.5             # exactly 1/32

F32 = mybir.dt.float32
F32R = mybir.dt.float32r
EXP = mybir.ActivationFunctionType.Exp

IC = N // 512                   # query chunks of 512 (2 matmul halves each)
JT = N // P                     # key tiles of 128
NJB = JT // 2                   # j-blocks of 2 key tiles


def build_nc(kt: int):
    """Build the single-core program (identical on all 8 cores).

    kt: number of 128-row contraction tiles for the qkv projection
        (8 for dim=1024, 9 when a ones-row block is appended to fold biases).
    """
    nc = bacc.Bacc(trn_type="TRN2")

    xT = nc.dram_tensor("xT", (kt * P, N), F32R, kind="ExternalInput")
    w = nc.dram_tensor("w", (kt * P, 3 * F_LOC), F32R, kind="ExternalInput")
    wo = nc.dram_tensor("wo", (F_LOC, DIM), F32R, kind="ExternalInput")
    out = nc.dram_tensor("out", (N, DIM), F32, kind="ExternalOutput")

    xT_t = xT[:].rearrange("(t p) n -> p t n", p=P)        # [128, kt, N]
    w_t = w[:].rearrange("(t p) f -> p t f", p=P)          # [128, kt, 768]
    wo_t = wo[:].rearrange("(t p) e -> p t e", p=P)        # [128, 2, 1024]

    with tile.TileContext(nc) as tc:
        with (
            tc.tile_pool(name="persist", bufs=1) as persist,
            tc.tile_pool(name="const", bufs=1) as const,
        ):
            qT = persist.tile([P, 2, N], F32R, tag="qT")     # [feat, ft, tok]
            kT = persist.tile([P, 2, N], F32R, tag="kT")
            # v interleaved with ones columns: slot 2h = v_h, slot 2h+1 = 1.0
            # so that lhsT = vo[:, jt, 2h:2h+2, :] is [v_h | ones] (M=128).
            vo = persist.tile([P, JT, 2 * H_LOC, DH], F32R, tag="vo")
            outT = persist.tile([P, 2, N], F32R, tag="outT")   # [hd, ktile, tok]
            wo_sb = persist.tile([P, 2, DIM], F32R, tag="wo")
            nc.gpsimd.memset(vo.bitcast(F32)[:, :, 1::2, :], 1.0)

            # ---- Phase 1: qkv projection ------------------------------
            with (
                tc.tile_pool(name="xt", bufs=2) as xt_pool,
                tc.tile_pool(name="wsb", bufs=1) as w_pool,
                tc.tile_pool(name="ps_qk", bufs=4, space="PSUM") as ps_qk,
                tc.tile_pool(name="ps_v", bufs=2, space="PSUM") as ps_v,
            ):
                w_sb = w_pool.tile([P, kt, 3 * F_LOC], F32R, tag="w")
                for k in range(kt):
                    nc.sync.dma_start(out=w_sb[:, k, :], in_=w_t[:, k, :])
                for c in range(IC):                       # token chunks of 512
                    csl = slice(c * 512, (c + 1) * 512)
                    xt = xt_pool.tile([P, kt, 512], F32R, tag="xt")
                    for k in range(kt):
                        nc.sync.dma_start(out=xt[:, k, :], in_=xT_t[:, k, csl])
                    # q^T and k^T (feature-major), N=256 halves
                    for which, dst in ((0, qT), (1, kT)):
                        for ft in range(2):
                            f0 = which * F_LOC + ft * P
                            ps = ps_qk.tile([P, 2, 256], F32, tag="psqk")
                            for ih in range(2):
                                for k in range(kt):
                                    # one accumulation group per PSUM bank:
                                    # start zeroes the whole 2KB bank, so
                                    # only the bank's first matmul starts it
                                    nc.tensor.matmul(
                                        ps[:, ih, :],
                                        lhsT=w_sb[:, k, f0:f0 + P],
                                        rhs=xt[:, k, ih * 256:(ih + 1) * 256],
                                        start=(ih == 0 and k == 0),
                                        stop=(ih == 1 and k == kt - 1),
                                        skip_group_check=True,
                                    )
                            nc.scalar.copy(out=dst[:, ft, csl], in_=ps)
                    # v (token-major), written into the even slots of vo
                    for tt in range(4):
                        psv = ps_v.tile([P, H_LOC, DH], F32, tag="psv")
                        for k in range(kt):
                            nc.tensor.matmul(
                                psv,
                                lhsT=xt[:, k, tt * P:(tt + 1) * P],
                                rhs=w_sb[:, k, 2 * F_LOC:3 * F_LOC],
                                start=(k == 0),
                                stop=(k == kt - 1),
                            )
                        nc.scalar.copy(out=vo[:, c * 4 + tt, 0::2, :], in_=psv)
                nc.sync.dma_start(out=wo_sb, in_=wo_t)

            # ---- Phase 2: attention -----------------------------------
            # N=512 moving operands; attn@v one j-block behind scores/exp.
            # avz tiles are double-buffered so the DVE normalize chain
            # (reciprocal+multiply, ~8us) never blocks the next chunk's
            # attn@v accumulation.
            with (
                tc.tile_pool(name="ps_s", bufs=2, space="PSUM") as ps_s,
                tc.tile_pool(name="ps_avz", bufs=2, space="PSUM") as ps_avz,
                tc.tile_pool(name="expp", bufs=4) as exp_pool,
                tc.tile_pool(name="rzp", bufs=4) as rz_pool,
            ):
                for ic in range(IC):
                    isl = slice(ic * 512, (ic + 1) * 512)
                    for pr in range(2):                   # head pair
                        # avz[h2][0:64] = attn@v head pr*2+h2 (unnormalized);
                        # avz[h2][64:128] = Z replicated (ones columns of vo)
                        avz = [
                            ps_avz.tile([P, 512], F32, tag=f"avz{h2}",
                                        name=f"avz{h2}_{ic}_{pr}")
                            for h2 in range(2)
                        ]
                        pend = None

                        def flush_avz(jb, exps):
                            for jt2 in range(2):
                                jt = jb * 2 + jt2
                                first = jb == 0 and jt2 == 0
                                last = jb == NJB - 1 and jt2 == 1
                                for h2 in range(2):
                                    h = pr * 2 + h2
                                    nc.tensor.matmul(
                                        avz[h2],
                                        lhsT=vo[:, jt, 2 * h:2 * h + 2, :],
                                        rhs=exps[h2][:, jt2, :],
                                        start=first,
                                        stop=last,
                                        skip_group_check=True,
                                    )

                        for jb in range(NJB):
                            ss = [ps_s.tile([P, 2, 512], F32, tag="s",
                                            name=f"s{ic}_{pr}_{jb}_{h2}")
                                  for h2 in range(2)]
                            for jt2 in range(2):
                                jt = jb * 2 + jt2
                                for h2 in range(2):
                                    hsl = slice(h2 * DH, (h2 + 1) * DH)
                                    nc.tensor.matmul(
                                        ss[h2][:, jt2, :],
                                        lhsT=kT[hsl, pr, jt * P:(jt + 1) * P],
                                        rhs=qT[hsl, pr, isl],
                                        start=True,
                                        stop=True,
                                        skip_group_check=True,
                                    )
                            exps = []
                            for h2 in range(2):
                                e = exp_pool.tile([P, 2, 512], F32R,
                                                  tag="exp",
                                                  name=f"e{ic}_{pr}_{jb}_{h2}")
                                nc.scalar.activation(e, ss[h2], EXP)
                                exps.append(e)
                            if pend is not None:
                                flush_avz(jb - 1, pend)
                            pend = exps
                        flush_avz(NJB - 1, pend)

                        # copy avz out of PSUM first (fast, releases the
                        # PSUM bank); the slow reciprocal then runs on the
                        # SBUF copy, off the PE/ACT critical path.
                        avzsb = []
                        for h2 in range(2):
                            t = rz_pool.tile([P, 512], F32, tag="avzsb",
                                             name=f"avzsb{h2}_{ic}_{pr}")
                            nc.vector.tensor_copy(t, avz[h2])
                            avzsb.append(t)
                        for h2 in range(2):
                            osl = slice(h2 * DH, (h2 + 1) * DH)
                            rz = rz_pool.tile([DH, 512], F32, tag="rz")
                            nc.vector.reciprocal(rz, avzsb[h2][DH:P, :])
                            nc.vector.tensor_mul(
                                out=outT[osl, pr, isl],
                                in0=avzsb[h2][0:DH, :],
                                in1=rz,
                            )

            # ---- Phase 3: output projection ---------------------------
            # dense PE tail; copies split between the (now idle) scalar
            # engine and DVE so neither gates the PE
            with (
                tc.tile_pool(name="ps_o", bufs=4, space="PSUM") as ps_o,
                tc.tile_pool(name="osb", bufs=8) as out_pool,
            ):
                u = 0
                for it in range(N // P):
                    i0 = it * P
                    for ec in range(2):
                        esl = slice(ec * 512, (ec + 1) * 512)
                        po = ps_o.tile([P, 512], F32, tag="po")
                        for kp in range(2):
                            nc.tensor.matmul(
                                po,
                                lhsT=outT[:, kp, i0:i0 + P],
                                rhs=wo_sb[:, kp, esl],
                                start=(kp == 0),
                                stop=(kp == 1),
                            )
                        po_sb = out_pool.tile([P, 512], F32, tag="po_sb",
                                              name=f"posb_{it}_{ec}")
                        if u % 2 == 0:
                            nc.scalar.copy(out=po_sb, in_=po)
                        else:
                            nc.vector.tensor_copy(po_sb, po)
                        u += 1
                        nc.sync.dma_start(out=out[i0:i0 + P, esl], in_=po_sb)
    nc.finalize()
    return nc


def _shard_inputs(x, w_qkv, b_qkv, w_out):
    """Host-side sharding: per-core input dicts (see module docstring)."""
    x = np.ascontiguousarray(x, dtype=np.float32)
    w_qkv = np.asarray(w_qkv, dtype=np.float32)
    b_qkv = np.asarray(b_qkv, dtype=np.float32)
    w_out = np.asarray(w_out, dtype=np.float32)

    has_bias = bool(np.any(b_qkv))
    kt = DIM // P + (1 if has_bias else 0)

    in_maps = []
    for c in range(NCORES):
        b = c // HGROUPS
        hg = c % HGROUPS
        fsl = slice(hg * F_LOC, (hg + 1) * F_LOC)
        # per-core weight shard [dim, 768]: q (pre-scaled), k, v columns
        w_shard = np.concatenate(
            [
                w_qkv[:, 0 * DIM:1 * DIM][:, fsl] * SCALE,
                w_qkv[:, 1 * DIM:2 * DIM][:, fsl],
                w_qkv[:, 2 * DIM:3 * DIM][:, fsl],
            ],
            axis=1,
        )
        xT_aug = np.zeros((kt * P, N), dtype=np.float32)
        xT_aug[:DIM] = x[b].T
        w_aug = np.zeros((kt * P, 3 * F_LOC), dtype=np.float32)
        w_aug[:DIM] = w_shard
        if has_bias:
            xT_aug[DIM] = 1.0
            w_aug[DIM] = np.concatenate(
                [
                    b_qkv[0 * DIM:1 * DIM][fsl] * SCALE,
                    b_qkv[1 * DIM:2 * DIM][fsl],
                    b_qkv[2 * DIM:3 * DIM][fsl],
                ]
            )
        in_maps.append(
            {
                "xT": np.ascontiguousarray(xT_aug),
                "w": np.ascontiguousarray(w_aug),
                "wo": np.ascontiguousarray(w_out[fsl, :]),
            }
        )
    return in_maps, kt


def _run(x, w_qkv, b_qkv, b_out, w_out, trace=False, **spmd_kwargs):
    in_maps, kt = _shard_inputs(x, w_qkv, b_qkv, w_out)
    nc = build_nc(kt)
    res = run_bass_kernel_spmd(
        nc, in_maps, core_ids=list(range(NCORES)), trace=trace, **spmd_kwargs
    )
    b_out = np.asarray(b_out, dtype=np.float32)
    full = np.empty((B, N, DIM), dtype=np.float32)
    for b in range(B):
        acc = res.results[b * HGROUPS]["out"].astype(np.float32)
        for hg in range(1, HGROUPS):
            acc = acc + res.results[b * HGROUPS + hg]["out"]
        full[b] = acc + b_out
    return full, res


def kernel(x, w_qkv, b_qkv, w_out, b_out):
    full, _ = _run(x, w_qkv, b_qkv, b_out, w_out, trace=False)
    return full



# revision 11
# speedup vs baseline: 1.9102x; 1.9102x over previous
"""Trainium2 Bass kernel: multi-head self-attention block (dense transformer).

Reference computation (fp32):
    qkv = x @ w_qkv + b_qkv                  # x [b, n, dim], w_qkv [dim, 3*dim]
    q, k, v = split(qkv); heads = 16, dh = 64
    dots = (q @ k^T) * dim**-0.5  (per head)
    attn = softmax(dots, axis=-1)
    out  = (attn @ v) @ w_out + b_out        # [b, n, dim]

Sharding (8 cores): data-parallel over batch (b=2) x tensor-parallel over
head-groups (4 groups of 4 heads).  core c -> batch c//4, head-group c%4.
Each core computes q/k/v for its 4 heads only, runs attention, and multiplies
by its 256-row slice of w_out, producing a partial [n, dim] output.  The host
sums the 4 partials per batch (the "all-reduce") and adds b_out.

v2 design (bf16 + engine-balance; ~2x over the fp32r v1):
  - ALL matmul operands are bf16 (fp32 PSUM accumulate).  fp32r pays an
    unhidden ~107ns LDWEIGHTS per matmul (no FWL for fp32); bf16 enables
    FWL so an N=512 matmul costs ~216ns vs ~330ns fp32r.  End-to-end bf16
    keeps global rel err ~5e-3 (gate 2e-2).
  - the scalar engine (ACT) runs exp at 1 elem/cycle/lane @1.2GHz: 16.8M
    score elements/core = ~110us busy -- a hard floor that rivals the PE
    (~140us).  So: ACT does ONLY exp (one FD=1024 instruction per key-tile,
    both heads of the pair in one [128,2,512] PSUM tile); every PSUM->SBUF
    copy runs on DVE; normalization uses reciprocal_approx_fast (5x faster
    than DVE reciprocal, 18-bit accurate) once per (chunk, pair).
  - software pipeline ACROSS the whole kernel, not per-phase: after the
    k-projection and first q-chunk land, scores+exp stream continuously;
    v-projection, remaining q-projection and the output projection are
    drip-fed into the PE stream as filler so the PE never idles while ACT
    works; attn@v trails exp by one (chunk, pair) unit (deep esb buffer).
  - scores are computed TRANSPOSED (S^T [j, i]) so attn@v needs no
    transpose; attn@v stationary = [v_h | ones] (M=128): rows 0-63 give the
    unnormalized attention output, rows 64-127 the softmax denominator Z
    replicated, so normalization is a plain DVE multiply.
  - PSUM budget (8 banks): scores tag [128,2,512]x2 = 4, avz [128,512]x2 = 2,
    shared qk/v/out-proj accumulator [128,512]x2 = 2.
"""

import numpy as np
import ml_dtypes

import concourse.bacc as bacc
import concourse.mybir as mybir
import concourse.tile as tile
from concourse.bass_utils import run_bass_kernel_spmd

P = 128
DIM = 1024
HEADS = 16
B = 2
N = 2048
NCORES = 8
HGROUPS = 4                     # head-groups (tensor parallel)
H_LOC = HEADS // HGROUPS        # 4 heads per core
DH = DIM // HEADS               # 64
F_LOC = H_LOC * DH              # 256 features per core (per q/k/v)
SCALE = DIM ** -0.5             # exactly 1/32

F32 = mybir.dt.float32
BF16 = mybir.dt.bfloat16
EXP = mybir.ActivationFunctionType.Exp
NPBF16 = ml_dtypes.bfloat16

IC = N // 512                   # query chunks of 512
JT = N // P                     # 16 key tiles of 128
NU = IC * 2                     # (chunk, head-pair) units


def build_nc(kt: int):
    """Build the single-core program (identical on all 8 cores).

    kt: number of 128-row contraction tiles for the qkv projection
        (8 for dim=1024, 9 when a ones-row block is appended to fold biases).
    """
    nc = bacc.Bacc(trn_type="TRN2")

    xT = nc.dram_tensor("xT", (kt * P, N), BF16, kind="ExternalInput")
    w = nc.dram_tensor("w", (kt * P, 3 * F_LOC), BF16, kind="ExternalInput")
    wo = nc.dram_tensor("wo", (F_LOC, DIM), BF16, kind="ExternalInput")
    out = nc.dram_tensor("out", (N, DIM), BF16, kind="ExternalOutput")

    xT_t = xT[:].rearrange("(t p) n -> p t n", p=P)        # [128, kt, N]
    w_t = w[:].rearrange("(t p) f -> p t f", p=P)          # [128, kt, 768]
    wo_t = wo[:].rearrange("(t p) e -> p t e", p=P)        # [128, 2, 1024]

    with tile.TileContext(nc) as tc:
        with (
            tc.tile_pool(name="persist", bufs=1) as persist,
            tc.tile_pool(name="esbp", bufs=18) as esbp,
            tc.tile_pool(name="normp", bufs=2) as normp,
            tc.tile_pool(name="outp", bufs=4) as outp,
            tc.tile_pool(name="psum", bufs=2, space="PSUM") as psum,
        ):
            x_sb = persist.tile([P, kt, N], BF16, tag="x")
            w_sb = persist.tile([P, kt, 3 * F_LOC], BF16, tag="w")
            qT = persist.tile([P, 2, N], BF16, tag="qT")     # [feat, ft, tok]
            kT = persist.tile([P, 2, N], BF16, tag="kT")
            # v interleaved with ones columns: slot 2h = v_h, slot 2h+1 = 1.0
            # so that lhsT = vo[:, jt, 2h:2h+2, :] is [v_h | ones] (M=128).
            vo = persist.tile([P, JT, 2 * H_LOC, DH], BF16, tag="vo")
            outT = persist.tile([P, 2, N], BF16, tag="outT")   # [hd, kp, tok]
            wo_sb = persist.tile([P, 2, DIM], BF16, tag="wo")
            nc.gpsimd.memset(vo[:, :, 1::2, :], 1.0)

            # PE warmup: the HAM clock gate keeps the PE at 1.2 GHz until
            # ~3.4us of sustained activity.  Burn that window on junk
            # matmuls over memset tiles while the input DMAs run, and
            # trigger the exp table load (~2.7us) early.
            wml = persist.tile([P, P], BF16, tag="wml")
            wmr = persist.tile([P, 512], BF16, tag="wmr")
            nc.gpsimd.memset(wml, 1.0)
            nc.gpsimd.memset(wmr, 1.0)
            wme = esbp.tile([P, 2, 512], BF16, tag="e", name="warm_e")
            nc.scalar.activation(wme[:, 0, 0:8], wmr[:, 0:8], EXP)

            def warm_mms(n, label):
                for g in range((n + 4) // 5):
                    ps = psum.tile([P, 512], F32, tag="acc",
                                   name=f"warm_{label}_{g}")
                    for i in range(min(5, n - g * 5)):
                        nc.tensor.matmul(ps, lhsT=wml, rhs=wmr,
                                         start=(i == 0), stop=True,
                                         skip_group_check=True)

            warm_mms(10, "head")

            # ---- input DMA, one batched transfer per section -----------
            def dma_w(c0, c1):
                nc.sync.dma_start(out=w_sb[:, :, c0:c1], in_=w_t[:, :, c0:c1])

            def dma_x(c):
                csl = slice(c * 512, (c + 1) * 512)
                nc.sync.dma_start(out=x_sb[:, :, csl], in_=xT_t[:, :, csl])

            dma_w(F_LOC, 2 * F_LOC)          # k columns
            dma_x(0)
            dma_x(1)
            dma_w(0, F_LOC)                  # q columns (pre-scaled)
            dma_x(2)
            dma_x(3)
            dma_w(2 * F_LOC, 3 * F_LOC)      # v columns
            nc.sync.dma_start(out=wo_sb, in_=wo_t)

            # ---- PE work generators ------------------------------------
            def qk_group(which, ft, c):
                """q/k projection: one [128 feat, 512 tok] accumulation."""
                csl = slice(c * 512, (c + 1) * 512)
                f0 = which * F_LOC + ft * P
                ps = psum.tile([P, 512], F32, tag="acc",
                               name=f"qk{which}_{ft}_{c}")
                for k in range(kt):
                    nc.tensor.matmul(
                        ps,
                        lhsT=w_sb[:, k, f0:f0 + P],
                        rhs=x_sb[:, k, csl],
                        start=(k == 0),
                        stop=(k == kt - 1),
                        skip_group_check=True,
                    )
                dst = qT if which == 0 else kT
                nc.vector.tensor_copy(dst[:, ft, csl], ps)

            def v_group(jt):
                """v projection: one [128 tok, 256 vfeat] accumulation."""
                tsl = slice(jt * P, (jt + 1) * P)
                ps = psum.tile([P, 512], F32, tag="acc", name=f"v{jt}")
                for k in range(kt):
                    nc.tensor.matmul(
                        ps[:, 0:F_LOC],
                        lhsT=x_sb[:, k, tsl],
                        rhs=w_sb[:, k, 2 * F_LOC:3 * F_LOC],
                        start=(k == 0),
                        stop=(k == kt - 1),
                        skip_group_check=True,
                    )
                nc.vector.tensor_copy(vo[:, jt, 0::2, :], ps[:, 0:F_LOC])

            def o_group(ic, it, ec):
                """output projection: [128 tok, 512 emb], K=256 (2 tiles)."""
                i0 = (ic * 4 + it) * P
                esl = slice(ec * 512, (ec + 1) * 512)
                po = psum.tile([P, 512], F32, tag="acc", name=f"po{ic}_{it}_{ec}")
                for kp in range(2):
                    nc.tensor.matmul(
                        po,
                        lhsT=outT[:, kp, i0:i0 + P],
                        rhs=wo_sb[:, kp, esl],
                        start=(kp == 0),
                        stop=(kp == 1),
                        skip_group_check=True,
                    )
                po_sb = outp.tile([P, 512], BF16, tag="po_sb",
                                  name=f"posb{ic}_{it}_{ec}")
                nc.vector.tensor_copy(po_sb, po)
                nc.sync.dma_start(out=out[i0:i0 + P, esl], in_=po_sb)

            # filler queue: drip-fed into the PE stream between attention
            # steps, throttled by an estimated PE-vs-ACT clock so the PE
            # stays just behind the exp stream (ACT must never starve for
            # scores). force() handles hard deadlines (deps of the next
            # attention step) regardless of budget.
            fillers = []                      # list of (key, emit_fn, est_us)
            emitted = set()
            clock = {"pe": 0.0, "act": 0.0}   # estimated engine timelines

            def push(key, fn, est):
                fillers.append((key, fn, est))

            def force(key):
                for i, (k2, fn, est) in enumerate(fillers):
                    if k2 == key:
                        fillers.pop(i)
                        emitted.add(key)
                        clock["pe"] += est
                        fn()
                        return
                assert key in emitted, f"missing filler {key}"

            def pop_budget(slack=0.3):
                while fillers and clock["pe"] < clock["act"] - slack:
                    key, fn, est = fillers.pop(0)
                    emitted.add(key)
                    clock["pe"] += est
                    fn()

            def pop_all():
                while fillers:
                    key, fn, est = fillers.pop(0)
                    emitted.add(key)
                    fn()

            QK_US, V_US, O_US = 1.75, 1.05, 0.5
            for c in range(1, IC):
                push(("k", 0, c), (lambda c=c: qk_group(1, 0, c)), QK_US)
            for c in range(IC):
                push(("k", 1, c), (lambda ft=1, c=c: qk_group(1, ft, c)), QK_US)
            push(("q", 1, 0), (lambda: qk_group(0, 1, 0)), QK_US)
            for jt in range(JT):
                push(("v", jt), (lambda jt=jt: v_group(jt)), V_US)
            for c in range(1, IC):
                push(("q", 0, c), (lambda c=c: qk_group(0, 0, c)), QK_US)
                push(("q", 1, c), (lambda c=c: qk_group(0, 1, c)), QK_US)

            # ---- attention pipeline ------------------------------------
            # unit u = (ic, pr): scores+exp for u stream in unit u; attn@v
            # for u-1 rides along one unit behind (esb holds the lag).
            avz = {}                          # (u, h2) -> psum tile
            esb = {}                          # (u, jt) -> sbuf exp tile

            def scores_step(u, jt):
                ic, pr = divmod(u, 2)
                isl = slice(ic * 512, (ic + 1) * 512)
                ss = psum.tile([P, 2, 512], F32, tag="s", name=f"s{u}_{jt}")
                for h2 in range(2):
                    hsl = slice(h2 * DH, (h2 + 1) * DH)
                    nc.tensor.matmul(
                        ss[:, h2, :],
                        lhsT=kT[hsl, pr, jt * P:(jt + 1) * P],
                        rhs=qT[hsl, pr, isl],
                        start=True,
                        stop=True,
                        skip_group_check=True,
                    )
                # flat views: a 2-D free AP over PSUM costs ~+400ns/inst
                e = esbp.tile([P, 2, 512], BF16, tag="e", name=f"e{u}_{jt}")
                nc.scalar.activation(e.rearrange("p a b -> p (a b)"),
                                     ss.rearrange("p a b -> p (a b)"), EXP)
                esb[(u, jt)] = e

            def attn_step(u, jt):
                for h2 in range(2):
                    if jt == 0:
                        avz[(u, h2)] = psum.tile(
                            [P, 512], F32, tag="avz", name=f"avz{u}_{h2}")
                    h = (u % 2) * 2 + h2
                    nc.tensor.matmul(
                        avz[(u, h2)],
                        lhsT=vo[:, jt, 2 * h:2 * h + 2, :],
                        rhs=esb[(u, jt)][:, h2, :],
                        start=(jt == 0),
                        stop=(jt == JT - 1),
                        skip_group_check=True,
                    )

            def norm_unit(u):
                """avz -> outT: copy out of PSUM, 1/Z, multiply (all DVE)."""
                ic, pr = divmod(u, 2)
                isl = slice(ic * 512, (ic + 1) * 512)
                azs = normp.tile([P, 2, 512], F32, tag="azs", name=f"azs{u}")
                for h2 in range(2):
                    nc.vector.tensor_copy(azs[:, h2, :], avz.pop((u, h2)))
                # reciprocal_approx_fast (custom DVE op) mishandles
                # partition-shifted in/out, so shift Z down to partitions
                # 0-63 with a plain copy first, then invert at same base.
                zt = normp.tile([DH, 2, 512], F32, tag="zt", name=f"zt{u}")
                nc.vector.tensor_copy(zt, azs[DH:P, :, :])
                rz = normp.tile([DH, 2, 512], F32, tag="rz", name=f"rz{u}")
                nc.vector.reciprocal_approx_fast(out=rz, in_=zt)
                for h2 in range(2):
                    nc.vector.tensor_mul(
                        out=outT[h2 * DH:(h2 + 1) * DH, pr, isl],
                        in0=azs[0:DH, h2, :],
                        in1=rz[:, h2, :],
                    )
                for jt in range(JT):
                    del esb[(u, jt)]

            # head: first k chunk + first q chunk; scores(u0) only need
            # kproj chunk jt//4, so the rest of the k projection streams
            # inside unit 0 right before the scores that consume it.
            qk_group(1, 0, 0)
            emitted.add(("k", 0, 0))
            qk_group(0, 0, 0)
            emitted.add(("q", 0, 0))
            clock["pe"] = 2 * QK_US
            clock["act"] = clock["pe"] + 3.0   # table load + first scores

            for u in range(NU):
                ic, pr = divmod(u, 2)
                # safety net: hard deps of this unit's scores
                if pr == 1:
                    for c in range(IC):
                        if ("k", 1, c) not in emitted:
                            force(("k", 1, c))
                if ("q", pr, ic) not in emitted:
                    force(("q", pr, ic))
                for jt in range(JT):
                    if u == 0:
                        if jt % 4 == 0 and jt > 0 and ("k", 0, jt // 4) not in emitted:
                            force(("k", 0, jt // 4))
                        if jt >= 8 and jt % 2 == 0 and ("k", 1, (jt - 8) // 2) not in emitted:
                            force(("k", 1, (jt - 8) // 2))
                    scores_step(u, jt)
                    clock["pe"] += 0.45
                    clock["act"] = max(clock["act"], clock["pe"]) + 1.2
                    if u > 0:
                        if ("v", jt) not in emitted:
                            force(("v", jt))
                        attn_step(u - 1, jt)
                        clock["pe"] += 0.45
                    if jt == 10 and u + 1 < NU:
                        # pre-emit the next unit's q projection so it
                        # doesn't stall the exp stream at the boundary
                        icn, prn = divmod(u + 1, 2)
                        if ("q", prn, icn) not in emitted:
                            force(("q", prn, icn))
                    pop_budget()
                if u > 0:
                    norm_unit(u - 1)
                    if u % 2 == 0:
                        icd = (u - 2) // 2
                        for it in range(4):
                            for ec in range(2):
                                push(("o", icd, it, ec),
                                     (lambda ic=icd, it=it, ec=ec:
                                      o_group(ic, it, ec)), O_US)

            # drain: last unit's attn@v, norm, out-proj; leftover fillers.
            for jt in range(JT):
                attn_step(NU - 1, jt)
            pop_all()
            norm_unit(NU - 1)
            warm_mms(12, "tail")     # keep the PE clock up through norm
            for it in range(4):
                for ec in range(2):
                    o_group(IC - 1, it, ec)
    nc.finalize()
    return nc


def _shard_inputs(x, w_qkv, b_qkv, w_out):
    """Host-side sharding: per-core input dicts (see module docstring)."""
    x = np.ascontiguousarray(x, dtype=np.float32)
    w_qkv = np.asarray(w_qkv, dtype=np.float32)
    b_qkv = np.asarray(b_qkv, dtype=np.float32)
    w_out = np.asarray(w_out, dtype=np.float32)

    has_bias = bool(np.any(b_qkv))
    kt = DIM // P + (1 if has_bias else 0)

    in_maps = []
    for c in range(NCORES):
        b = c // HGROUPS
        hg = c % HGROUPS
        fsl = slice(hg * F_LOC, (hg + 1) * F_LOC)
        # per-core weight shard [dim, 768]: q (pre-scaled), k, v columns
        w_shard = np.concatenate(
            [
                w_qkv[:, 0 * DIM:1 * DIM][:, fsl] * SCALE,
                w_qkv[:, 1 * DIM:2 * DIM][:, fsl],
                w_qkv[:, 2 * DIM:3 * DIM][:, fsl],
            ],
            axis=1,
        )
        xT_aug = np.zeros((kt * P, N), dtype=np.float32)
        xT_aug[:DIM] = x[b].T
        w_aug = np.zeros((kt * P, 3 * F_LOC), dtype=np.float32)
        w_aug[:DIM] = w_shard
        if has_bias:
            xT_aug[DIM] = 1.0
            w_aug[DIM] = np.concatenate(
                [
                    b_qkv[0 * DIM:1 * DIM][fsl] * SCALE,
                    b_qkv[1 * DIM:2 * DIM][fsl],
                    b_qkv[2 * DIM:3 * DIM][fsl],
                ]
            )
        in_maps.append(
            {
                "xT": np.ascontiguousarray(xT_aug).astype(NPBF16),
                "w": np.ascontiguousarray(w_aug).astype(NPBF16),
                "wo": np.ascontiguousarray(w_out[fsl, :]).astype(NPBF16),
            }
        )
    return in_maps, kt


def _run(x, w_qkv, b_qkv, b_out, w_out, trace=False, **spmd_kwargs):
    in_maps, kt = _shard_inputs(x, w_qkv, b_qkv, w_out)
    nc = build_nc(kt)
    res = run_bass_kernel_spmd(
        nc, in_maps, core_ids=list(range(NCORES)), trace=trace, **spmd_kwargs
    )
    b_out = np.asarray(b_out, dtype=np.float32)
    full = np.empty((B, N, DIM), dtype=np.float32)
    for b in range(B):
        acc = res.results[b * HGROUPS]["out"].astype(np.float32)
        for hg in range(1, HGROUPS):
            acc = acc + res.results[b * HGROUPS + hg]["out"].astype(np.float32)
        full[b] = acc + b_out
    return full, res


def kernel(x, w_qkv, b_qkv, w_out, b_out):
    full, _ = _run(x, w_qkv, b_qkv, b_out, w_out, trace=False)
    return full
